# revision 1
# baseline (speedup 1.0000x reference)
"""AlternatingHighwayLSTM Trainium2 (Bass) kernel - 8-core SPMD.

Self-contained: builds the Bass program (once, cached), shards the batch
axis across 8 NeuronCores, runs via run_bass_kernel_spmd, reassembles.

Strategy: data-parallel over batch (B=64 -> 8 per core), weights replicated.
Per layer: hoisted input GEMM (fp32r) staged through DRAM, then the 256-step
recurrence with a K=9 identity+bias PSUM inject, 4x K=128 fp32r matmuls
against the transposed hidden state, gates on ScalarE straight from PSUM,
cell/highway update on VectorE/GpSimd, and accumulating PE-transposes that
sum the three highway terms in PSUM while writing the next stationary
directly into the following layer's input buffer.

Note: the dropout mask is algebraically folded into (o*m) and (lin*m), so the
produced per-step state equals h*m; for the problem's inputs (eval mode,
mask == 1) this is exactly h.
"""
import numpy as np
import concourse.bass as bass
import concourse.mybir as mybir
import concourse.tile as tile
from concourse.bass_utils import run_bass_kernel_spmd
from concourse.masks import make_identity


# ---- wait-split post-pass ----


_DMA_OPS = {"DMACopy", "TensorLoad", "TensorSave", "DMATransposeAnt", "TriggerDMA"}


def split_excess_waits(nc, limit=1, dma_limit=1, verbose=False):
    n_split = 0
    uid = [0]
    for f in nc.m.functions:
        for b in f.blocks:
            out = []
            changed = False
            for ins in b.instructions:
                si = ins.sync_info
                lim = dma_limit if ins.opcode in _DMA_OPS else limit
                if si is not None and si.on_wait is not None and len(si.on_wait) > lim:
                    waits = list(si.on_wait)
                    extra, keep = waits[:-lim], waits[-lim:]
                    for w in extra:
                        ev = mybir.InstEventSemaphore(
                            name=f"WSPLIT-{uid[0]}")
                        uid[0] += 1
                        ev.engine = ins.engine
                        ev.sync_info = mybir.SyncInfo(on_wait=[w], on_update=[])
                        out.append(ev)
                        n_split += 1
                    ins.sync_info = mybir.SyncInfo(
                        on_wait=keep, on_update=list(si.on_update or []))
                    changed = True
                out.append(ins)
            if changed:
                try:
                    b.instructions = out
                except Exception:
                    # fall back: clear+extend if assignment unsupported
                    b.instructions.clear()
                    b.instructions.extend(out)
    if verbose:
        print(f"split_excess_waits: hoisted {n_split} waits")
    return n_split

# ---- kernel builder ----

import numpy as np
import concourse.bass as bass
import concourse.mybir as mybir
import concourse.tile as tile
from concourse.masks import make_identity

F32 = mybir.dt.float32
F32R = mybir.dt.float32r
AF = mybir.ActivationFunctionType
OP = mybir.AluOpType

H = 512
G5 = 5 * H          # 2560
G6 = 6 * H          # 3072
NBUF = 3            # stage prefetch depth
B = 8               # per-core batch


def weight_offsets(L=8, IN=1024):
    offs = []
    o = 0
    for l in range(L):
        in_l = IN if l == 0 else H
        offs.append((o, o + in_l * G6))
        o += in_l * G6 + H * G5
    return offs  # (wx_off, wh_off) per layer


def build_full(S=256, L=8, IN=1024):
    nc = bass.Bass("TRN2", target_bir_lowering=False, debug=False)
    T = S * B  # tokens per layer

    x_d = nc.dram_tensor("inputs", [S, B, IN], F32, kind="ExternalInput").ap()
    w_d = nc.dram_tensor("weight", [24641536], F32, kind="ExternalInput").ap()
    b_d = nc.dram_tensor("bias", [20480], F32, kind="ExternalInput").ap()
    m_d = nc.dram_tensor("dropout_mask", [8, B, H], F32, kind="ExternalInput").ap()
    o_d = nc.dram_tensor("out", [S, B, H], F32, kind="ExternalOutput").ap()
    proj_d = [nc.dram_tensor(f"proj{i}", [T, G6], F32).ap() for i in range(2)]

    offs = weight_offsets(L=8, IN=IN)

    with tile.TileContext(nc) as tc:
        with (
            tc.tile_pool(name="sb", bufs=1) as sb,
            tc.tile_pool(name="ps", bufs=1, space="PSUM") as ps,
        ):
            # ---------------- persistent tiles ----------------
            ident = sb.tile([128, 128], F32, tag="ident")
            make_identity(nc, ident[:, :])
            lhsT9_t = sb.tile([9, 8], F32R, tag="lhsT9")
            # build identity+ones inject stationary in fp32 scratch, bounce it
            # through DRAM so the fp32r tile's writer is a DMA
            lhsT9_scratch = sb.tile([9, 8], F32, tag="lhsT9s")
            lhsT9_dram = nc.dram_tensor("lhsT9_dram", [9, 8], F32).ap()
            nc.vector.memset(lhsT9_scratch[:, :], 0.0)
            nc.gpsimd.affine_select(
                out=lhsT9_scratch[:, :], in_=lhsT9_scratch[:, :],
                compare_op=OP.not_equal, fill=1.0, base=0,
                pattern=[[-1, 8]], channel_multiplier=1)
            nc.gpsimd.affine_select(
                out=lhsT9_scratch[:, :], in_=lhsT9_scratch[:, :],
                compare_op=OP.not_equal, fill=1.0, base=-8,
                pattern=[[0, 8]], channel_multiplier=1)
            nc.sync.dma_start(out=lhsT9_dram, in_=lhsT9_scratch[:, :])
            nc.sync.dma_start(out=lhsT9_t[:, :], in_=lhsT9_dram.bitcast(F32R))

            # big shared weight slot: wx halves (48KB) / wh (40KB)
            wslot = sb.tile([128, 12288], F32R, tag="wslot")
            wh_t = wslot[:, 0:4 * G5]  # (128, 10240) view for recurrence

            # hT_seq ping/pong: (128, 4 chunks x T tokens) fp32r
            hseq = [sb.tile([128, 4 * T], F32R, tag=f"hseq{i}", name=f"hseq{i}")
                    for i in range(2)]

            stages = [sb.tile([9, G6], F32R, tag=f"stage{i}", name=f"stage{i}")
                      for i in range(NBUF)]

            mask_t = sb.tile([32, H], F32, tag="mask")
            c_t = sb.tile([32, H], F32, tag="c")
            h_t = [sb.tile([32, H], F32, tag=f"h{i}", name=f"h{i}")
                   for i in range(2)]
            gates = [sb.tile([32, H], F32, tag=f"gate{g}", name=f"gate{g}")
                     for g in range(5)]
            t_t = sb.tile([32, H], F32, tag="t")
            u_t = sb.tile([32, H], F32, tag="u")
            th_t = sb.tile([32, H], F32, tag="th")
            u2_t = sb.tile([32, H], F32, tag="u2")
            w_t = sb.tile([32, H], F32, tag="w")
            om_t = sb.tile([32, H], F32, tag="om")
            linm_t = sb.tile([32, H], F32, tag="linm")
            linmn_t = sb.tile([32, H], F32, tag="linmn")
            bq_t = sb.tile([32, H], F32, tag="bq")
            maskn_t = sb.tile([32, H], F32, tag="maskn")
            e7_t = sb.tile([32, H], F32, tag="e7")

            # psum tiles: 5 z banks + tr + 2 proj banks = 8
            z = [ps.tile([B, H], F32, tag=f"z{g}", name=f"z{g}") for g in range(5)]
            # proj psum / layer0-xpose / recurrence h-transpose slots share a
            # 3-slot tag (5 z banks + 3 pp banks = 8)

            def pp_tile(name):
                return ps.tile([128, 512], F32, tag="pp", bufs=3, name=name)

            # layer-0 x staging (slots shared with gate tiles: disjoint lifetimes)
            x0_t = [sb.tile([128, IN], F32, tag=f"gate{i}", name=f"x0_{i}")
                    for i in range(2)]
            xT0_t = [sb.tile([128, IN], F32R, tag=f"gate{i+2}", name=f"xT0_{i}")
                     for i in range(2)]
            pstage = [sb.tile([128, 512], F32, tag=f"pst{i}", name=f"pst{i}")
                      for i in range(3)]
            # one-time zero fill of the stage row-8 lin sections via DMA bounce
            nc.vector.memset(pstage[0][0:1, :], 0.0)
            for i in range(NBUF):
                nc.sync.dma_start(out=stages[i][8:9, G5:G6],
                                  in_=pstage[0][0:1, :].bitcast(F32R))

            # ---------------- per layer ----------------
            for l in range(L):
                in_l = IN if l == 0 else H
                KC = in_l // 128
                wx_off, wh_off = offs[l]
                pd = proj_d[l % 2]
                MT = T // 128  # token chunks

                # ---- proj GEMM over two n-halves of 1536 cols ----
                for half in range(2):
                    ncol0 = 1536 * half
                    # load wx half: KC chunks x 1536 cols into wslot
                    for k in range(KC):
                        src = w_d[wx_off + k * 128 * G6:
                                  wx_off + (k + 1) * 128 * G6]
                        srcv = src.rearrange("(r c) -> r c", c=G6)
                        nc.sync.dma_start(
                            out=wslot[:, 1536 * k:1536 * (k + 1)],
                            in_=srcv[:, ncol0:ncol0 + 1536].bitcast(F32R))
                    for m in range(MT):
                        if l == 0:
                            xm = x0_t[m % 2]
                            xTm = xT0_t[m % 2]
                            xsrc = x_d.rearrange("s b i -> (s b) i")
                            nc.sync.dma_start(
                                out=xm[:, :],
                                in_=xsrc[128 * m:128 * (m + 1), :])
                            for k in range(KC):
                                xp = pp_tile(f"xp_{l}_{half}_{m}_{k}")
                                nc.tensor.transpose(
                                    xp[:, 0:128],
                                    xm[:, 128 * k:128 * (k + 1)],
                                    ident[:, :])
                                nc.vector.tensor_copy(
                                    xTm[:, 128 * k:128 * (k + 1)],
                                    xp[:, 0:128])

                            def lhsT(k, m=m, xTm=xTm):
                                return xTm[:, 128 * k:128 * (k + 1)]
                        else:
                            hs = hseq[(l - 1) % 2]

                            def lhsT(k, m=m, hs=hs):
                                return hs[:, T * k + 128 * m:
                                          T * k + 128 * (m + 1)]
                        for n3 in range(3):
                            ppt = pp_tile(f"pp_{l}_{half}_{m}_{n3}")
                            for k in range(KC):
                                nc.tensor.matmul(
                                    ppt[:, :], lhsT(k),
                                    wslot[:, 1536 * k + 512 * n3:
                                          1536 * k + 512 * (n3 + 1)],
                                    start=(k == 0), stop=(k == KC - 1))
                            pst = pstage[(m * 3 + n3) % 3]
                            # ScalarE is idle during proj; keep DVE free and
                            # let psum-slot turnaround pace below the MM rate
                            nc.scalar.copy(pst[:, :], ppt[:, :])
                            nc.sync.dma_start(
                                out=pd[128 * m:128 * (m + 1),
                                       ncol0 + 512 * n3:ncol0 + 512 * (n3 + 1)],
                                in_=pst[:, :])

                # ---- recurrence ----
                # load wh into wslot (after proj consumed wx)
                for k in range(4):
                    src = w_d[wh_off + k * 128 * G5:wh_off + (k + 1) * 128 * G5]
                    nc.sync.dma_start(
                        out=wh_t[:, k * G5:(k + 1) * G5],
                        in_=src.rearrange("(r c) -> r c", c=G5).bitcast(F32R))
                nc.sync.dma_start(out=mask_t[0:B, :], in_=m_d[l, :, :])
                nc.vector.tensor_scalar_mul(maskn_t[0:B, :], mask_t[0:B, :], -1.0)
                nc.vector.memset(c_t[0:B, :], 0.0)
                bias_l = b_d[G5 * l:G5 * (l + 1)].rearrange("(o c) -> o c", o=1)
                for i in range(NBUF):
                    nc.sync.dma_start(out=stages[i][8:9, 0:G5],
                                      in_=bias_l.bitcast(F32R))

                def tok(s):
                    return s if l % 2 == 0 else S - 1 - s

                def stage_dma(s):
                    nc.sync.dma_start(
                        out=stages[s % NBUF][0:8, :],
                        in_=pd[B * tok(s):B * (tok(s) + 1), :].bitcast(F32R))

                # bank order [g i f o r]; weight col order [i f g o r]
                COL = {0: 2, 1: 0, 2: 1, 3: 3, 4: 4}
                hs_out = hseq[l % 2]

                def emit_injects(s):
                    st = stages[s % NBUF]
                    for g in range(5):
                        cb = COL[g]
                        nc.tensor.matmul(z[g][:, :], lhsT9_t[:, :],
                                         st[:, H * cb:H * (cb + 1)],
                                         start=True, stop=(s == 0))

                def emit_kmms(s):
                    # stationary: hT of step s-1 lives in hs_out at token tok(s-1)
                    tp = tok(s - 1)
                    for g in range(5):
                        cb = COL[g]
                        for k in range(4):
                            nc.tensor.matmul(
                                z[g][:, :],
                                hs_out[:, T * k + B * tp:T * k + B * (tp + 1)],
                                wh_t[:, G5 * k + H * cb:G5 * k + H * (cb + 1)],
                                start=False, stop=(k == 3))

                def emit_sigs(s):
                    for g in range(4):
                        func = AF.Tanh if g == 0 else AF.Sigmoid
                        nc.scalar.activation(gates[g][0:B, :], z[g][:, :], func)
                    nc.scalar.activation(gates[4][0:B, :], z[4][:, :], AF.Sigmoid)

                def emit_chain(s):
                    # h = w' + lin*m - r*lin*m with w' = (r*(o*m))*th; the
                    # three terms are summed in PSUM by accumulating PE
                    # transposes, so no per-chunk DVE work remains.
                    st = stages[s % NBUF]
                    g_t, i_t, f_t, o_t, r_t = gates
                    lin = st[0:B, G5:G6].bitcast(F32)
                    nc.vector.tensor_tensor(linm_t[0:B, :], lin,
                                            mask_t[0:B, :], OP.mult)
                    nc.gpsimd.tensor_tensor(linmn_t[0:B, :], lin,
                                            maskn_t[0:B, :], OP.mult)
                    nc.vector.tensor_tensor(t_t[0:B, :], i_t[0:B, :],
                                            g_t[0:B, :], OP.mult)
                    nc.vector.tensor_tensor(u_t[0:B, :], f_t[0:B, :],
                                            c_t[0:B, :], OP.mult)
                    nc.vector.tensor_tensor(om_t[0:B, :], o_t[0:B, :],
                                            mask_t[0:B, :], OP.mult)
                    nc.vector.tensor_tensor(c_t[0:B, :], u_t[0:B, :],
                                            t_t[0:B, :], OP.add)
                    nc.scalar.activation(th_t[0:B, 0:256], c_t[0:B, 0:256],
                                         AF.Tanh)
                    nc.scalar.activation(th_t[0:B, 256:H], c_t[0:B, 256:H],
                                         AF.Tanh)
                    # q = (o*m)*th ready before sig_r; w'_ck = r*q_ck after
                    for k2 in range(4):
                        sl = slice(128 * k2, 128 * (k2 + 1))
                        nc.vector.tensor_tensor(u2_t[0:B, sl], om_t[0:B, sl],
                                                th_t[0:B, sl], OP.mult)
                    # B' = r * (-lin*m), in chunks on gpsimd
                    for k2 in range(4):
                        sl = slice(128 * k2, 128 * (k2 + 1))
                        nc.gpsimd.tensor_tensor(bq_t[0:B, sl], r_t[0:B, sl],
                                                linmn_t[0:B, sl], OP.mult)
                    tko = tok(s)
                    for k2 in range(4):
                        sl = slice(128 * k2, 128 * (k2 + 1))
                        nc.vector.tensor_tensor(w_t[0:B, sl], r_t[0:B, sl],
                                                u2_t[0:B, sl], OP.mult)
                        hp = pp_tile(f"ht_{l}_{s}_{k2}")
                        nc.tensor.matmul(hp[:, 0:8], linm_t[0:B, sl],
                                         ident[0:B, 0:B], is_transpose=True,
                                         start=True, stop=False)
                        nc.tensor.matmul(hp[:, 0:8], bq_t[0:B, sl],
                                         ident[0:B, 0:B], is_transpose=True,
                                         start=False, stop=False)
                        nc.tensor.matmul(hp[:, 0:8], w_t[0:B, sl],
                                         ident[0:B, 0:B], is_transpose=True,
                                         start=False, stop=True)
                        nc.scalar.copy(
                            hs_out[:, T * k2 + B * tko:
                                   T * k2 + B * (tko + 1)],
                            hp[:, 0:8])
                    if l == L - 1:
                        hn = h_t[s % 2]
                        nc.vector.tensor_tensor(e7_t[0:B, :], w_t[0:B, :],
                                                linm_t[0:B, :], OP.add)
                        nc.vector.tensor_tensor(hn[0:B, :], e7_t[0:B, :],
                                                bq_t[0:B, :], OP.add)
                        nc.sync.dma_start(out=o_d[tok(s), :, :],
                                          in_=hn[0:B, :])

                for s in range(min(NBUF, S)):
                    stage_dma(s)
                emit_injects(0)
                for s in range(S):
                    if s > 0:
                        emit_kmms(s)
                    emit_sigs(s)
                    if s + 1 < S:
                        emit_injects(s + 1)
                    emit_chain(s)
                    if s + NBUF < S:
                        stage_dma(s + NBUF)
    return nc


_CACHE = {}


def _get_nc():
    if "nc" not in _CACHE:
        nc = build_full(S=256, L=8, IN=1024)
        split_excess_waits(nc)
        _CACHE["nc"] = nc
    return _CACHE["nc"]


def hw_exec_time_estimate_ns():
    if "tl" not in _CACHE:
        from concourse.timeline_sim import TimelineSim
        _CACHE["tl"] = int(TimelineSim(_get_nc(), trace=False).simulate())
    return _CACHE["tl"]


def kernel(inputs, weight, bias, dropout_mask):
    inputs = np.ascontiguousarray(inputs, dtype=np.float32)
    weight = np.ascontiguousarray(weight, dtype=np.float32)
    bias = np.ascontiguousarray(bias, dtype=np.float32)
    dropout_mask = np.ascontiguousarray(dropout_mask, dtype=np.float32)
    nc = _get_nc()
    n_cores = 8
    in_maps = []
    for i in range(n_cores):
        sl = slice(8 * i, 8 * (i + 1))
        in_maps.append({
            "inputs": inputs[:, sl, :],
            "weight": weight,
            "bias": bias,
            "dropout_mask": dropout_mask[:, sl, :],
        })
    res = run_bass_kernel_spmd(nc, in_maps, list(range(n_cores)))
    out = np.concatenate([res.results[i]["out"] for i in range(n_cores)], axis=1)
    return np.ascontiguousarray(out, dtype=np.float32)

